# revision 3
# baseline (speedup 1.0000x reference)
"""AttentionFlowLayer (BiDAF-style) Trainium2 kernel, 8 NeuronCores.

Sharding: data-parallel over batch N=16 -> 2 batches per core, weights
replicated, no collectives.

Per batch (Lc=2048, Lq=256, D=256), per 128-row context tile:
  S'[i,j] = sum_d c[i,d]*w_m[d]*q[j,d] + qw[j]          (bf16 matmul, f32 psum)
  col 256 of the same psum = cw[i] = c_i . w_c           (extra rhs column)
  A' = exp(S')  (ScalarE, accum_out -> Z_i row sums)     row softmax numerator
  c2q[i,:] = (A' @ q) / Z_i
  eb[i] = rowmax(A') * exp(cw[i])  (= exp(max_j S[i,j])) q2c softmax numerator
  q2c = (sum_i eb_i * c16[i,:]) / sum_i eb_i             (matmul accumulation)
  G tile = [c, c2q, c*c2q, c*q2c] in bf16, DMA'd out; host upcasts to f32.
"""

import numpy as np

N, LC, LQ, D = 16, 2048, 256, 256
NCORES = 8
NB = N // NCORES      # batches per core
P = 128
T = LC // P           # context tiles per batch
JT = LQ // P          # query partition tiles
DC = D // P           # d chunks

_cache = {}


def _build():
    import concourse.mybir as mybir
    from concourse import bacc
    from concourse.tile import TileContext
    from concourse.masks import make_identity

    f32 = mybir.dt.float32
    bf16 = mybir.dt.bfloat16
    EXP = mybir.ActivationFunctionType.Exp
    COPY = mybir.ActivationFunctionType.Copy
    AX = mybir.AxisListType.X

    nc = bacc.Bacc("TRN2")
    c_in = nc.dram_tensor("emb_context", (NB, LC, D), f32, kind="ExternalInput")
    q_in = nc.dram_tensor("emb_query", (NB, LQ, D), f32, kind="ExternalInput")
    w_in = nc.dram_tensor("W", (3 * D,), f32, kind="ExternalInput")
    out = nc.dram_tensor("out", (NB, LC, 4 * D), bf16, kind="ExternalOutput")

    with TileContext(nc) as tc:
        with (
            tc.tile_pool(name="const", bufs=1) as constp,
            tc.tile_pool(name="qpool", bufs=2) as qpool,
            tc.tile_pool(name="cin", bufs=3) as cinp,
            tc.tile_pool(name="ct", bufs=3) as ctp,
            tc.tile_pool(name="apool", bufs=3) as app,
            tc.tile_pool(name="aptp", bufs=3) as aptp,
            tc.tile_pool(name="gstag", bufs=2 * T + 4) as gp,
            tc.tile_pool(name="small", bufs=6) as smallp,
            tc.tile_pool(name="batchp", bufs=2) as bp,
            tc.tile_pool(name="ps_s", bufs=2, space="PSUM") as ps_s,
            tc.tile_pool(name="ps_t", bufs=2, space="PSUM") as ps_t,
            tc.tile_pool(name="ps_cq", bufs=2, space="PSUM") as ps_cq,
            tc.tile_pool(name="ps_sm", bufs=1, space="PSUM") as ps_sm,
        ):
            ident = constp.tile([P, P], bf16, tag="ident")
            make_identity(nc, ident)
            ones_row = constp.tile([1, P], bf16, tag="ones_row")
            nc.vector.memset(ones_row, 1.0)
            ones_col = constp.tile([P, 1], bf16, tag="ones_col")
            nc.vector.memset(ones_col, 1.0)
            # W columns: [wc0 wc1 wq0 wq1 wm0 wm1], chunk c covers d=c*128..c*128+127
            wcols = constp.tile([P, 6], f32, tag="wcols")
            nc.sync.dma_start(wcols, w_in[:].rearrange("(c p) -> p c", p=P))
            wq16 = constp.tile([P, 2], bf16, tag="wq16")
            nc.vector.tensor_copy(wq16, wcols[:, 2:4])

            for b in range(NB):
                # ---- per-batch query prep ----
                qf = qpool.tile([P, JT, D], f32, tag="qf")
                nc.sync.dma_start(qf, q_in[b].rearrange("(jt p) d -> p jt d", p=P))
                q16 = qpool.tile([P, JT, D], bf16, tag="q16")
                nc.vector.tensor_copy(q16, qf)
                # qT16[p, c, j] = q16[j, c*128+p]
                qT16 = qpool.tile([P, DC, LQ], bf16, tag="qT16")
                for c in range(DC):
                    pst = ps_t.tile([P, LQ], bf16, tag="pst")
                    for jt in range(JT):
                        nc.tensor.transpose(
                            pst[:, jt * P:(jt + 1) * P],
                            q16[:, jt, c * P:(c + 1) * P],
                            ident,
                        )
                    nc.vector.tensor_copy(qT16[:, c, :], pst)
                # qmTx[:, c, 0:LQ] = qT16 * w_m[c];  col LQ = w_c[c]
                qmTx = qpool.tile([P, DC, LQ + 1], bf16, tag="qmTx")
                for c in range(DC):
                    nc.vector.tensor_scalar_mul(
                        qmTx[:, c, 0:LQ], qT16[:, c, :], wcols[:, 4 + c:5 + c]
                    )
                    nc.vector.tensor_copy(qmTx[:, c, LQ:LQ + 1], wcols[:, c:c + 1])
                # qw row: qw[j] = q_j . w_q ; col LQ stays 0
                ps_qw = ps_sm.tile([1, LQ], f32, tag="sm")
                for c in range(DC):
                    nc.tensor.matmul(
                        ps_qw,
                        lhsT=wq16[:, c:c + 1],
                        rhs=qT16[:, c, :],
                        start=(c == 0),
                        stop=(c == DC - 1),
                    )
                qwx = qpool.tile([1, LQ + 1], bf16, tag="qwx")
                nc.vector.memset(qwx, 0.0)
                nc.vector.tensor_copy(qwx[:, 0:LQ], ps_qw)

                ebstag = bp.tile([P, T], f32, tag="ebstag")
                gts = []
                # ---- pass 1: per context tile ----
                for t in range(T):
                    cf = cinp.tile([P, D], f32, tag="cf")
                    nc.sync.dma_start(cf, c_in[b, t * P:(t + 1) * P, :])
                    gt = gp.tile([P, 4 * D], bf16, tag="gstag")
                    gts.append(gt)
                    # chunk0: c in bf16 (also feeds matmuls / muls)
                    nc.gpsimd.tensor_copy(gt[:, 0:D], cf)
                    # cT16[p, c*128+i'] = c16[i', c*128+p]
                    pst = ps_t.tile([P, D], bf16, tag="pst")
                    for c in range(DC):
                        nc.tensor.transpose(
                            pst[:, c * P:(c + 1) * P], gt[:, c * P:(c + 1) * P], ident
                        )
                    cT16 = ctp.tile([P, D], bf16, tag="ct16")
                    nc.vector.tensor_copy(cT16, pst)
                    # S' psum
                    ps_S_t = ps_s.tile([P, LQ + 1], f32, tag="ps_s")
                    for c in range(DC):
                        nc.tensor.matmul(
                            ps_S_t,
                            lhsT=cT16[:, c * P:(c + 1) * P],
                            rhs=qmTx[:, c, :],
                            start=(c == 0),
                            stop=False,
                        )
                    nc.tensor.matmul(
                        ps_S_t, lhsT=ones_row, rhs=qwx, start=False, stop=True
                    )
                    # A' = exp(S'), Z = row sums
                    Ap_t = app.tile([P, LQ], bf16, tag="ap")
                    Zi = smallp.tile([P, 1], f32, tag="zi")
                    nc.scalar.activation(Ap_t, ps_S_t[:, 0:LQ], EXP, accum_out=Zi)
                    rowmax = smallp.tile([P, 1], f32, tag="rmax")
                    nc.vector.reduce_max(rowmax, Ap_t, axis=AX)
                    ecw = smallp.tile([P, 1], f32, tag="ecw")
                    nc.scalar.activation(ecw, ps_S_t[:, LQ:LQ + 1], EXP)
                    nc.vector.tensor_mul(ebstag[:, t:t + 1], rowmax, ecw)
                    invZ = smallp.tile([P, 1], f32, tag="invz")
                    nc.vector.reciprocal(invZ, Zi)
                    # A'T
                    psa = ps_t.tile([P, LQ], bf16, tag="pst")
                    for jt in range(JT):
                        nc.tensor.transpose(
                            psa[:, jt * P:(jt + 1) * P],
                            Ap_t[:, jt * P:(jt + 1) * P],
                            ident,
                        )
                    ApT = aptp.tile([P, LQ], bf16, tag="apt")
                    nc.vector.tensor_copy(ApT, psa)
                    # c2q = (A' @ q) / Z
                    ps_c2q_t = ps_cq.tile([P, D], f32, tag="cq")
                    for jt in range(JT):
                        nc.tensor.matmul(
                            ps_c2q_t,
                            lhsT=ApT[:, jt * P:(jt + 1) * P],
                            rhs=q16[:, jt, :],
                            start=(jt == 0),
                            stop=(jt == JT - 1),
                        )
                    nc.scalar.activation(gt[:, D:2 * D], ps_c2q_t, COPY, scale=invZ)
                    # chunk2 = c * c2q
                    nc.vector.tensor_mul(gt[:, 2 * D:3 * D], gt[:, 0:D], gt[:, D:2 * D])
                    nc.sync.dma_start(
                        out[b, t * P:(t + 1) * P, 0:3 * D], gt[:, 0:3 * D]
                    )

                # ---- batch finalize: q2c ----
                ebrow = smallp.tile([P, 1], f32, tag="ebrow")
                nc.vector.reduce_sum(ebrow, ebstag, axis=AX)
                eb16 = bp.tile([P, T], bf16, tag="eb16")
                nc.vector.tensor_copy(eb16, ebstag)
                ebrow16 = smallp.tile([P, 1], bf16, tag="ebrow16")
                nc.vector.tensor_copy(ebrow16, ebrow)
                ps_zb = ps_sm.tile([1, 1], f32, tag="sm")
                nc.tensor.matmul(ps_zb, lhsT=ebrow16, rhs=ones_col, start=True, stop=True)
                zb = smallp.tile([1, 1], f32, tag="zb")
                nc.vector.tensor_copy(zb, ps_zb)
                inv_zb = smallp.tile([1, 1], f32, tag="invzb")
                nc.vector.reciprocal(inv_zb, zb)
                ps_q2c = ps_sm.tile([1, D], f32, tag="sm")
                for t in range(T):
                    nc.tensor.matmul(
                        ps_q2c,
                        lhsT=eb16[:, t:t + 1],
                        rhs=gts[t][:, 0:D],
                        start=(t == 0),
                        stop=(t == T - 1),
                    )
                q2cn16 = smallp.tile([1, D], bf16, tag="q2cn")
                nc.scalar.activation(q2cn16, ps_q2c, COPY, scale=inv_zb)
                # broadcast q2c to 128 partitions
                ps_bc = ps_cq.tile([P, D], f32, tag="cq")
                nc.tensor.matmul(ps_bc, lhsT=ones_row, rhs=q2cn16, start=True, stop=True)
                q2cb16 = bp.tile([P, D], bf16, tag="q2cb")
                nc.vector.tensor_copy(q2cb16, ps_bc)
                # ---- pass 2: chunk3 = c * q2c ----
                for t in range(T):
                    nc.vector.tensor_mul(
                        gts[t][:, 3 * D:4 * D], gts[t][:, 0:D], q2cb16
                    )
                    nc.sync.dma_start(
                        out[b, t * P:(t + 1) * P, 3 * D:4 * D], gts[t][:, 3 * D:4 * D]
                    )

    nc.compile()
    return nc


def _get_nc():
    if "nc" not in _cache:
        _cache["nc"] = _build()
    return _cache["nc"]


def run(emb_context, emb_query, W, trace=False, **kwargs):
    from concourse.bass_utils import run_bass_kernel_spmd

    nc = _get_nc()
    emb_context = np.asarray(emb_context, dtype=np.float32)
    emb_query = np.asarray(emb_query, dtype=np.float32)
    W = np.asarray(W, dtype=np.float32)
    in_maps = [
        {
            "emb_context": np.ascontiguousarray(emb_context[c * NB:(c + 1) * NB]),
            "emb_query": np.ascontiguousarray(emb_query[c * NB:(c + 1) * NB]),
            "W": W,
        }
        for c in range(NCORES)
    ]
    res = run_bass_kernel_spmd(
        nc, in_maps, core_ids=list(range(NCORES)), trace=trace, **kwargs
    )
    outs = [np.asarray(r["out"], dtype=np.float32) for r in res.results]
    return np.concatenate(outs, axis=0), res


def kernel(emb_context, emb_query, W):
    out, _ = run(emb_context, emb_query, W, trace=False)
    return out
